# revision 13
# baseline (speedup 1.0000x reference)
"""EntMaxSelectLayer distributed Trainium2 kernel (v2).

Computes out = x @ entmax15(weight, axis=-1) with
  x [512, 8192] f32, weight [8192, 4096] f32, out [512, 4096] f32.

Strategy (8 NeuronCores, SPMD):
  - weight row-sharded: core d gets rows [1024d, 1024d+1024) (8 tiles of
    [128, 4096]); x column-sharded and host-cast to bf16 (xT [1024, 512]).
  - entmax15 per weight row computed exactly via top-64 candidates
    (union of per-512-chunk DVE top-8) + sort-based threshold recursion;
    engine-balanced: DVE does max8/sort, GpSimd the [128,64] stat chain
    and the final square, Scalar does sqrt + relu reconstruction.
  - matmul x.T-shard @ p in two contraction phases so the PE works while
    weights stream in: phase A accumulates tiles 0-3 into PSUM and spills
    bf16 to SBUF (scalar copies); phase B reloads each spill into PSUM
    with an identity matmul and accumulates tiles 4-7 on top.
  - partial [512, 4096] bf16 written as two column halves; each half is
    ReduceScattered (CCE adds in the SDMA path) as soon as it completes,
    overlapping the first collective with the rest of the matmul.
    Core r gets rows [64r, 64r+64) of the sum; cast f32 and store.
"""

import numpy as np

B, IN, OUT = 512, 8192, 4096
NCORES = 8
ROWS = IN // NCORES          # 1024 weight rows per core
NT = ROWS // 128             # 8 weight tiles of [128, 4096] per core
T = 32                       # top-k length for the exact mini-entmax
NEG_FILL = -1e30
HALF = OUT // 2              # 2048-wide collective chunks

_cache = {}


def _build_program():
    from concourse import bacc, mybir, tile
    from concourse.alu_op_type import AluOpType

    f32 = mybir.dt.float32
    bf16 = mybir.dt.bfloat16

    nc = bacc.Bacc(
        "TRN2",
        target_bir_lowering=False,
        debug=False,
        enable_asserts=False,
        num_devices=NCORES,
    )

    w_ext = nc.dram_tensor("w", [ROWS, OUT], f32, kind="ExternalInput")
    xT_ext = nc.dram_tensor("xT", [ROWS, B], bf16, kind="ExternalInput")
    out_ext = nc.dram_tensor("out", [B // NCORES, OUT], f32, kind="ExternalOutput")

    rg = [list(range(NCORES))]

    with tile.TileContext(nc) as tc:
        with (
            tc.tile_pool(name="consts", bufs=1) as cpool,
            tc.tile_pool(name="wpool", bufs=3) as wpool,
            tc.tile_pool(name="ppool", bufs=NT) as ppool,
            tc.tile_pool(name="xpool", bufs=1) as xpool,
            tc.tile_pool(name="spill", bufs=4) as lpool,
            tc.tile_pool(name="small", bufs=2) as spool,
            tc.tile_pool(name="psum", bufs=8, space="PSUM") as psum_pool,
            tc.tile_pool(name="evac", bufs=4) as epool,
            tc.tile_pool(name="dram", bufs=1, space="DRAM") as dpool,
            tc.tile_pool(name="fin", bufs=2) as fpool,
        ):
            # ---- constants ----
            iota1 = cpool.tile([128, T], f32)
            nc.gpsimd.iota(
                iota1[:], [[1, T]], base=1, channel_multiplier=0,
                allow_small_or_imprecise_dtypes=True,
            )
            rinv = cpool.tile([128, T], f32)
            nc.vector.reciprocal(rinv[:], iota1[:])
            zero64 = cpool.tile([128, T], f32)
            nc.gpsimd.memset(zero64[:], 0.0)
            # identity [128, 128] bf16 for PSUM reload-matmuls
            idtmp = cpool.tile([128, 128], f32)
            nc.gpsimd.iota(
                idtmp[:], [[1, 128]], base=0, channel_multiplier=0,
                allow_small_or_imprecise_dtypes=True,
            )
            pidx = cpool.tile([128, 1], f32)
            nc.gpsimd.iota(
                pidx[:], [[1, 1]], base=0, channel_multiplier=1,
                allow_small_or_imprecise_dtypes=True,
            )
            ident = cpool.tile([128, 128], bf16)
            nc.vector.tensor_scalar(
                ident[:], idtmp[:], pidx[:], None, AluOpType.is_equal
            )

            # ---- input DMAs ----
            # xT arrives as bf16 [1024, 512]; lay out so column block i holds
            # rows [128i, 128i+128) => lhsT slice for (tile i, batch blk b).
            xT_sb = xpool.tile([128, NT * B], bf16, name="xT_sb")
            xT_v = xT_ext.ap().rearrange("(t p) b -> p t b", p=128)

            w_tiles = []
            for t in range(NT):
                wt = wpool.tile([128, OUT], bf16, name=f"wt{t}", tag="wt")
                for hh in range(2):
                    nc.gpsimd.dma_start(
                        out=wt[:, 2048 * hh:2048 * (hh + 1)],
                        in_=w_ext.ap()[128 * t:128 * (t + 1),
                                       2048 * hh:2048 * (hh + 1)],
                    )
                w_tiles.append(wt)
                if t == 0:
                    nc.sync.dma_start(
                        out=xT_sb[:].rearrange("p (t b) -> p t b", t=NT), in_=xT_v
                    )

            p_tiles = [None] * NT
            spillA = [None] * 4   # per-batch-block [128, OUT] bf16 (tiles 0-3 sum)

            def entmax_tile(t):
                wt = w_tiles[t]
                # candidates: top-8 of each 512-wide chunk (DVE, bf16 2x)
                cand = spool.tile([128, 64], bf16, tag="cand", name=f"cand{t}")
                for c in range(8):
                    nc.vector.max(cand[:, 8 * c:8 * c + 8], wt[:, 512 * c:512 * (c + 1)])
                # sorted top-32 (descending) of candidates (DVE)
                v64 = spool.tile([128, T], bf16, tag="v64", name=f"v64{t}")
                for j in range(T // 8):
                    nc.vector.max(v64[:, 8 * j:8 * j + 8], cand[:])
                    if j < T // 8 - 1:
                        nc.vector.match_replace(
                            cand[:], v64[:, 8 * j:8 * j + 8], cand[:], NEG_FILL
                        )
                m32 = spool.tile([128, 1], f32, tag="m32", name=f"m32{t}")
                nc.gpsimd.tensor_copy(m32[:], v64[:, 0:1])
                m_ap = m32[:]  # row max (f32)

                # ---- threshold recursion on [128, T] ----
                zs = spool.tile([128, T], f32, tag="zs", name=f"zs{t}")
                nc.vector.tensor_scalar(
                    zs[:], v64[:], m_ap, 0.5, AluOpType.subtract, AluOpType.mult
                )
                zsq = spool.tile([128, T], f32, tag="zsq", name=f"zsq{t}")
                nc.gpsimd.tensor_tensor(zsq[:], zs[:], zs[:], AluOpType.mult)
                cs1 = spool.tile([128, T], f32, tag="cs1", name=f"cs1{t}")
                nc.vector.tensor_tensor_scan(
                    cs1[:], zs[:], zero64[:], 0.0, AluOpType.add, AluOpType.add
                )
                cs2 = spool.tile([128, T], f32, tag="cs2", name=f"cs2{t}")
                nc.vector.tensor_tensor_scan(
                    cs2[:], zsq[:], zero64[:], 0.0, AluOpType.add, AluOpType.add
                )
                mean = spool.tile([128, T], f32, tag="mean", name=f"mean{t}")
                nc.gpsimd.tensor_tensor(mean[:], cs1[:], rinv[:], AluOpType.mult)
                msq = spool.tile([128, T], f32, tag="msq", name=f"msq{t}")
                nc.gpsimd.tensor_tensor(msq[:], cs2[:], rinv[:], AluOpType.mult)
                meansq = spool.tile([128, T], f32, tag="meansq", name=f"meansq{t}")
                nc.gpsimd.tensor_tensor(meansq[:], mean[:], mean[:], AluOpType.mult)
                # delta = (1 - rho*(msq - mean^2)) / rho = (rinv - msq) + mean^2
                delta = spool.tile([128, T], f32, tag="delta", name=f"delta{t}")
                nc.gpsimd.tensor_tensor(delta[:], rinv[:], msq[:], AluOpType.subtract)
                nc.gpsimd.tensor_tensor(delta[:], delta[:], meansq[:], AluOpType.add)
                nc.gpsimd.tensor_scalar(
                    delta[:], delta[:], 0.0, None, AluOpType.max
                )
                sq = spool.tile([128, T], f32, tag="sq", name=f"sq{t}")
                nc.scalar.activation(sq[:], delta[:], mybir.ActivationFunctionType.Sqrt)
                tau = spool.tile([128, T], f32, tag="tau", name=f"tau{t}")
                nc.gpsimd.tensor_tensor(tau[:], mean[:], sq[:], AluOpType.subtract)
                # support = sum(tau <= zs); tau_star = tau[support - 1]
                cond = spool.tile([128, T], f32, tag="cond", name=f"cond{t}")
                supp = spool.tile([128, 1], f32, tag="supp", name=f"supp{t}")
                nc.vector.tensor_tensor(cond[:], tau[:], zs[:], AluOpType.is_le)
                nc.vector.tensor_reduce(
                    supp[:], cond[:], mybir.AxisListType.X, AluOpType.add
                )
                issel = spool.tile([128, T], f32, tag="issel", name=f"issel{t}")
                nc.vector.tensor_scalar(
                    issel[:], iota1[:], supp[:], None, AluOpType.is_equal
                )
                tsel = spool.tile([128, T], f32, tag="tsel", name=f"tsel{t}")
                tau_star = spool.tile([128, 1], f32, tag="tau_star", name=f"taus{t}")
                nc.gpsimd.tensor_tensor(tsel[:], tau[:], issel[:], AluOpType.mult)
                nc.vector.tensor_reduce(
                    tau_star[:], tsel[:], mybir.AxisListType.X, AluOpType.add
                )
                # negc = -(0.5*m + tau_star) = (m * -0.5) - tau_star
                negc = spool.tile([128, 1], f32, tag="negc", name=f"negc{t}")
                nc.vector.tensor_scalar(
                    negc[:], m_ap, -0.5, tau_star[:],
                    AluOpType.mult, AluOpType.subtract,
                )
                # r = relu(0.5*w + negc) (Scalar, frees wt), p = r*r (GpSimd)
                p = ppool.tile([128, OUT], bf16, tag="p", name=f"p{t}")
                nc.scalar.activation(
                    p[:], wt[:], mybir.ActivationFunctionType.Relu,
                    bias=negc[:], scale=0.5,
                )
                nc.scalar.activation(
                    p[:, 0:2048], p[:, 0:2048],
                    mybir.ActivationFunctionType.Square,
                )
                nc.gpsimd.tensor_tensor(
                    p[:, 2048:4096], p[:, 2048:4096], p[:, 2048:4096],
                    AluOpType.mult,
                )
                p_tiles[t] = p

            def lhsT(i, b):
                return xT_sb[:, 512 * i + 128 * b:512 * i + 128 * (b + 1)]

            def phaseA_stage(b):
                # accumulate tiles 0-3 for batch block b over all 8 kq chunks,
                # spill to SBUF bf16 via scalar copies
                ps = [
                    psum_pool.tile([128, 512], f32, tag="ps", name=f"psA{b}_{kq}")
                    for kq in range(8)
                ]
                for i in range(4):
                    for kq in range(8):
                        nc.tensor.matmul(
                            ps[kq][:],
                            lhsT=lhsT(i, b),
                            rhs=p_tiles[i][:, 512 * kq:512 * (kq + 1)],
                            start=(i == 0),
                            stop=(i == 3),
                            skip_group_check=True,
                        )
                sp = lpool.tile([128, OUT], bf16, tag="spill", name=f"spill{b}")
                for kq in range(8):
                    nc.scalar.copy(sp[:, 512 * kq:512 * (kq + 1)], ps[kq][:])
                spillA[b] = sp

            # partial column halves (Local internal DRAM: collective inputs)
            partial = [
                dpool.tile([B, HALF], bf16, name=f"partial{h}") for h in range(2)
            ]
            rs_out = [
                dpool.tile([B // NCORES, HALF], bf16, name=f"rs_out{h}")
                for h in range(2)
            ]

            def phaseB_group(h, bpair):
                # restore spills into PSUM via identity matmul, accumulate
                # tiles 4-7, evacuate bf16 and write partial half h
                chunks = [(b, kq) for b in bpair for kq in range(4)]
                ps = [
                    psum_pool.tile([128, 512], f32, tag="ps", name=f"psB{h}_{b}_{kq}")
                    for (b, kq) in chunks
                ]
                for j, (b, kq) in enumerate(chunks):
                    nc.tensor.matmul(
                        ps[j][:],
                        lhsT=ident[:],
                        rhs=spillA[b][:, 2048 * h + 512 * kq:2048 * h + 512 * (kq + 1)],
                        start=True,
                        stop=False,
                        skip_group_check=True,
                    )
                for i in range(4, 8):
                    for j, (b, kq) in enumerate(chunks):
                        nc.tensor.matmul(
                            ps[j][:],
                            lhsT=lhsT(i, b),
                            rhs=p_tiles[i][:, 2048 * h + 512 * kq:2048 * h + 512 * (kq + 1)],
                            start=False,
                            stop=(i == 7),
                            skip_group_check=True,
                        )
                for j, (b, kq) in enumerate(chunks):
                    ev = epool.tile([128, 512], bf16, tag="ev", name=f"ev{h}_{b}_{kq}")
                    if j % 2 == 0:
                        nc.scalar.copy(ev[:], ps[j][:])
                    else:
                        nc.vector.tensor_copy(ev[:], ps[j][:])
                    nc.sync.dma_start(
                        out=partial[h][128 * b:128 * (b + 1), 512 * kq:512 * (kq + 1)],
                        in_=ev[:],
                    )

            def finalize(h):
                fs = fpool.tile([B // NCORES, HALF], bf16, tag="fs", name=f"fs{h}")
                nc.sync.dma_start(out=fs[:], in_=rs_out[h][:, :])
                ff = fpool.tile([B // NCORES, HALF], f32, tag="ff", name=f"ff{h}")
                nc.vector.tensor_copy(ff[:], fs[:])
                nc.sync.dma_start(
                    out=out_ext.ap()[:, HALF * h:HALF * (h + 1)], in_=ff[:]
                )

            # ---- program order: interleave entmax with phase A stages so
            # scalar evacuations land between relu's, and phase B groups fire
            # each ReduceScatter as soon as its column half completes ----
            for t in range(4):
                entmax_tile(t)
            phaseA_stage(0)
            entmax_tile(4)
            phaseA_stage(1)
            entmax_tile(5)
            phaseA_stage(2)
            entmax_tile(6)
            phaseA_stage(3)
            entmax_tile(7)

            for h in range(2):
                phaseB_group(h, (0, 1))
                phaseB_group(h, (2, 3))
                nc.gpsimd.collective_compute(
                    "ReduceScatter",
                    mybir.AluOpType.add,
                    replica_groups=rg,
                    ins=[partial[h].opt()],
                    outs=[rs_out[h].opt()],
                )
            for h in range(2):
                finalize(h)

    nc.compile()
    return nc


def get_program():
    if "nc" not in _cache:
        _cache["nc"] = _build_program()
    return _cache["nc"]


def kernel(x: np.ndarray, weight: np.ndarray, trace: bool = False):
    import ml_dtypes
    from concourse.bass_utils import run_bass_kernel_spmd

    x = np.ascontiguousarray(x, dtype=np.float32)
    weight = np.ascontiguousarray(weight, dtype=np.float32)
    assert x.shape == (B, IN) and weight.shape == (IN, OUT)

    nc = get_program()
    in_maps = []
    for d in range(NCORES):
        in_maps.append({
            "w": np.ascontiguousarray(weight[ROWS * d:ROWS * (d + 1), :]),
            "xT": np.ascontiguousarray(
                x[:, ROWS * d:ROWS * (d + 1)].T.astype(ml_dtypes.bfloat16)
            ),
        })
    res = run_bass_kernel_spmd(
        nc, in_maps, core_ids=list(range(NCORES)), trace=trace
    )
    out = np.concatenate(
        [res.results[d]["out"] for d in range(NCORES)], axis=0
    )
    if trace:
        _cache["last_result"] = res
    return out
